# revision 7
# baseline (speedup 1.0000x reference)
"""Trainium2 Bass kernel for nn_Encoder_Postnet (alignment walk + gather).

Contract: kernel(**inputs) takes the FULL unsharded inputs
(encoder_out [32,512,512] f32, align_phone [32,4096] int, text_phone
[32,512] int) and returns the FULL output [32,4096,512] f32, matching

    vmap(_align_one)(encoder_out, align_phone, text_phone)

where _align_one is a sequential walk producing per-frame encoder-row
indices followed by a row gather.

Strategy: the walk itself is tiny integer work (B*T_ALIGN steps) and is
computed on host; the memory-bound part — materializing the 268 MB
gathered output — runs on 8 NeuronCores, batch-sharded 4 elems/core.

Device programs:
  * fast path (input has the uniform duration-expanded structure, i.e.
    idx[t] == t//d for all t and every frame valid): each core loads its
    4 encoder slices into SBUF once (4 MB) and writes the d-times
    row-replicated output with static strided DMAs. HBM traffic is
    read-once + write-once = ~37.5 MB/core, i.e. the memory roofline.
  * generic path (any other walk result): indirect-DMA row gather
    (128 rows per descriptor batch) through SBUF, double buffered.
"""

import numpy as np

from concourse import bass, mybir
from concourse.bass_utils import run_bass_kernel_spmd

B, T_TEXT, T_ALIGN, D = 32, 512, 4096, 512
N_CORES = 8
BPC = B // N_CORES  # batch elems per core
P = 128
J = T_TEXT // P  # free-dim slots per partition for one encoder slice

_PROGRAM_CACHE = {}


# ---------------------------------------------------------------- host scan
def _host_scan(align, text):
    """Replicates the reference jax.lax.scan walk in numpy, vectorized
    over batch. align [B, T_ALIGN] int32, text [B, T_TEXT] int32.
    Returns idx [B, T_ALIGN] int32, valid [B, T_ALIGN] bool."""
    Bn, Ta = align.shape
    Tt = text.shape[1]
    idx = np.zeros((Bn, Ta), np.int32)
    valid = np.ones((Bn, Ta), np.bool_)
    ind = np.zeros(Bn, np.int32)
    before = text[:, 0].copy()
    done = np.zeros(Bn, np.bool_)
    rows = np.arange(Bn)
    for t in range(1, Ta):
        a = align[:, t]
        match = a == before
        inc = np.where(match, ind, ind + 1)
        overflow = inc >= Tt
        new_done = done | (~match & overflow)
        safe = np.minimum(inc, Tt - 1)
        new_before = np.where(match | new_done, before, text[rows, safe])
        new_ind = np.where(new_done, ind, inc)
        idx[:, t] = new_ind
        valid[:, t] = ~new_done
        ind, before, done = new_ind, new_before, new_done
    return idx, valid


def _is_uniform_expansion(align, text):
    """True iff the walk provably yields idx[t] = t // d with every frame
    valid: align is exactly text repeated d times per phone and adjacent
    text phones are distinct (so each run boundary advances by exactly 1
    and the index never overflows)."""
    if T_ALIGN % T_TEXT:
        return False
    d = T_ALIGN // T_TEXT
    if not np.array_equal(align, np.repeat(text, d, axis=1)):
        return False
    return bool(np.all(text[:, 1:] != text[:, :-1]))


# ------------------------------------------------------------- fast program
def _build_fast(d):
    """Per-core program: enc [BPC,T_TEXT,D] -> out [BPC,T_ALIGN,D] where
    out[b, t] = enc[b, t // d]. Loads enc into SBUF once; each store DMA
    covers one (batch elem, j-slot): the SBUF source repeats each row d
    times via a stride-0 dim, so the DRAM write is d*D elems (16 KB)
    contiguous per partition."""
    nc = bass.Bass()
    f32 = mybir.dt.float32
    enc = nc.dram_tensor("enc", [BPC, T_TEXT, D], f32, kind="ExternalInput")
    out = nc.dram_tensor("out", [BPC, T_ALIGN, D], f32, kind="ExternalOutput")
    FREE = BPC * J * D  # free elems per partition in enc_sb

    with (
        nc.sbuf_tensor("enc_sb", [P, BPC, J, D], f32) as enc_sb,
        nc.semaphore("l0") as l0,
        nc.semaphore("l1") as l1,
        nc.semaphore("l2") as l2,
        nc.semaphore("l3") as l3,
        nc.semaphore("ss") as ss,
        nc.Block() as block,
    ):
        lsem = [l0, l1, l2, l3]

        # Loads b0/b1 issue immediately in parallel (one per HWDGE ring);
        # each ring then stores with odd/even replicas split across rings,
        # with the remaining loads slotted in between early stores so
        # every load completes well before its stores come up.
        def enc_src(b):
            return enc[b].rearrange("(p j) e -> p j e", j=J)

        def ov(b):
            return out[b].rearrange("(p j d) e -> p j d e", p=P, j=J)

        @block.gpsimd
        def _(gpsimd):
            for b in range(BPC):
                gpsimd.wait_ge(lsem[b], 16)
                for k in (2, 5):
                    gpsimd.dma_start(
                        ov(b)[:, :, k, :], enc_sb[:, b, :, :]
                    ).then_inc(ss, 16)

        @block.scalar
        def _(scalar):
            scalar.dma_start(enc_sb[:, 1, :, :], enc_src(1)).then_inc(lsem[1], 16)
            scalar.dma_start(enc_sb[:, 3, :, :], enc_src(3)).then_inc(lsem[3], 16)
            for b in range(BPC):
                scalar.wait_ge(lsem[b], 16)
                for k in (1, 4, 7):
                    scalar.dma_start(
                        ov(b)[:, :, k, :], enc_sb[:, b, :, :]
                    ).then_inc(ss, 16)

        @block.sync
        def _(sync):
            sync.dma_start(enc_sb[:, 0, :, :], enc_src(0)).then_inc(lsem[0], 16)
            sync.dma_start(enc_sb[:, 2, :, :], enc_src(2)).then_inc(lsem[2], 16)
            for b in range(BPC):
                sync.wait_ge(lsem[b], 16)
                for k in (0, 3, 6):
                    sync.dma_start(
                        ov(b)[:, :, k, :], enc_sb[:, b, :, :]
                    ).then_inc(ss, 16)
            sync.wait_ge(ss, BPC * d * 16)

    return nc


# ---------------------------------------------------------- generic program
def _build_generic():
    """Per-core program: flat encoder table enc [BPC*(T_TEXT+1), D] (one
    zero row appended per batch elem), per-frame row indices idx
    [P, BPC*NCH] (host pre-transposed, already offset into the flat
    table, invalid frames pointed at the zero row) -> out [BPC,T_ALIGN,D].
    Gathers 128 rows per indirect DMA, double buffered through SBUF."""
    nc = bass.Bass()
    f32 = mybir.dt.float32
    i32 = mybir.dt.int32
    NCH = T_ALIGN // P  # index chunks per batch elem
    enc = nc.dram_tensor("enc", [BPC * (T_TEXT + 1), D], f32, kind="ExternalInput")
    idxt = nc.dram_tensor("idx", [P, BPC * NCH], i32, kind="ExternalInput")
    out = nc.dram_tensor("out", [BPC, T_ALIGN, D], f32, kind="ExternalOutput")
    NBUF = 4

    with (
        nc.sbuf_tensor("idx_sb", [P, BPC * NCH], i32) as idx_sb,
        nc.sbuf_tensor("row_sb", [P, NBUF, D], f32) as row_sb,
        nc.semaphore("isem") as isem,
        nc.semaphore("gsem") as gsem,
        nc.semaphore("wsem") as wsem,
        nc.Block() as block,
    ):

        @block.gpsimd
        def _(gpsimd):
            gpsimd.dma_start(idx_sb[:], idxt[:]).then_inc(isem, 16)
            gpsimd.wait_ge(isem, 16)
            for i in range(BPC * NCH):
                buf = i % NBUF
                if i >= NBUF:
                    # wait until the store that reads this buffer is done
                    gpsimd.wait_ge(wsem, (i - NBUF + 1) * 16)
                gpsimd.indirect_dma_start(
                    out=row_sb[:, buf, :],
                    out_offset=None,
                    in_=enc[:],
                    in_offset=bass.IndirectOffsetOnAxis(
                        ap=idx_sb[:, i : i + 1], axis=0
                    ),
                ).then_inc(gsem, 16)

        @block.sync
        def _(sync):
            for i in range(BPC * NCH):
                b, c = divmod(i, NCH)
                buf = i % NBUF
                sync.wait_ge(gsem, (i + 1) * 16)
                sync.dma_start(
                    out[b, c * P : (c + 1) * P, :], row_sb[:, buf, :]
                ).then_inc(wsem, 16)
            sync.wait_ge(wsem, BPC * NCH * 16)

    return nc


# ------------------------------------------------------------------- driver
def _run(encoder_out, align_phone, text_phone, trace=False):
    enc = np.ascontiguousarray(np.asarray(encoder_out, dtype=np.float32))
    align = np.asarray(align_phone).astype(np.int32)
    text = np.asarray(text_phone).astype(np.int32)

    if _is_uniform_expansion(align, text):
        d = T_ALIGN // T_TEXT
        key = ("fast", d)
        if key not in _PROGRAM_CACHE:
            _PROGRAM_CACHE[key] = _build_fast(d)
        nc = _PROGRAM_CACHE[key]
        in_maps = [{"enc": enc[i * BPC : (i + 1) * BPC]} for i in range(N_CORES)]
    else:
        idx, valid = _host_scan(align, text)
        # route invalid frames at the per-batch zero pad row, flatten the
        # batch into the row index so the indirect table offset is 0
        idx_safe = np.where(valid, idx, T_TEXT).astype(np.int32)
        idx_safe += (np.arange(B, dtype=np.int32) * (T_TEXT + 1))[:, None]
        enc_pad = np.concatenate(
            [enc, np.zeros((B, 1, D), np.float32)], axis=1
        )  # [B, T_TEXT+1, D]
        NCH = T_ALIGN // P
        if "generic" not in _PROGRAM_CACHE:
            _PROGRAM_CACHE["generic"] = _build_generic()
        nc = _PROGRAM_CACHE["generic"]
        in_maps = []
        for i in range(N_CORES):
            sl = slice(i * BPC, (i + 1) * BPC)
            enc_i = enc_pad[sl].reshape(BPC * (T_TEXT + 1), D)
            # idx for this core, rebased to core-local flat rows, laid out
            # [P, BPC*NCH] so partition p holds row index for frame c*P+p
            idx_i = idx_safe[sl] - i * BPC * (T_TEXT + 1)
            idx_i = np.ascontiguousarray(
                idx_i.reshape(BPC, NCH, P).transpose(2, 0, 1).reshape(P, BPC * NCH)
            )
            in_maps.append(
                {"enc": np.ascontiguousarray(enc_i), "idx": idx_i}
            )

    res = run_bass_kernel_spmd(nc, in_maps, list(range(N_CORES)), trace=trace)
    full = np.concatenate([r["out"] for r in res.results], axis=0)
    return full, res


def kernel(encoder_out, align_phone, text_phone):
    full, _ = _run(encoder_out, align_phone, text_phone)
    return full
